# revision 31
# baseline (speedup 1.0000x reference)
"""GQA attention (32 q heads / 8 kv heads, L=2048, D=2048, hd=64) on 8 TRN2 cores.

Tensor-parallel over heads: core c owns q heads 4c..4c+3 and kv head c.
Per-core: project q/k/v (bf16 matmuls, f32 accum, k+v packed into one
full-width pass), RoPE via cos/sin tables + pair-swap matmul, causal-skipped
transposed-score attention (exp on ScalarE, softmax denominator from an
augmented ones-column in V), two half-AllGathers of the (l, 256)-per-core
attention output overlapped with attention/wo, then a d-sharded wo matmul.
Scores matmuls for mb-block pairs run concurrently in disjoint PE row groups
(K=64 each) via duplicated q/k partition halves. All pre-wo phases share one
flat 8-bank PSUM budget so projection, RoPE and attention interleave freely.
Host glue shards/transposes inputs and assembles the (out, k, v) tuple.
"""

import sys

import numpy as np

sys.path.insert(0, "/opt/trn_rl_repo")

import ml_dtypes  # noqa: E402

import concourse.tile as tile  # noqa: E402
from concourse import bacc, mybir  # noqa: E402
from concourse.bass_utils import run_bass_kernel_spmd  # noqa: E402
from concourse.masks import make_identity  # noqa: E402

P = 128
L = 2048
D = 2048
HD = 64          # head dim
HQ = 4           # q heads per core
OQ = 256         # q rows per core
OKV = 64         # kv rows per core
NB = 16          # 128-blocks along d / l / o
CH = 512         # l-chunk (PSUM bank)
NCH = 4
HL = L // 2
NCORES = 8
SCALE = HD ** -0.5
FP = mybir.dt.float32
BF = mybir.dt.bfloat16
BF_NP = ml_dtypes.bfloat16

_CACHE = {}


def _build():
    nc = bacc.Bacc(
        "TRN2", target_bir_lowering=False, debug=False, num_devices=NCORES
    )
    xT = nc.dram_tensor("xT", [D, L], BF, kind="ExternalInput").ap()
    wqT = nc.dram_tensor("wqT", [D, OQ], BF, kind="ExternalInput").ap()
    wkT = nc.dram_tensor("wkT", [D, OKV], BF, kind="ExternalInput").ap()
    wvT = nc.dram_tensor("wvT", [D, OKV], BF, kind="ExternalInput").ap()
    woT = nc.dram_tensor("woT", [D, OQ], BF, kind="ExternalInput").ap()
    ctab = nc.dram_tensor("ctab", [P, L], FP, kind="ExternalInput").ap()
    stab = nc.dram_tensor("stab", [P, L], FP, kind="ExternalInput").ap()
    pswap = nc.dram_tensor("pswap", [P, P], FP, kind="ExternalInput").ap()
    out_oT = nc.dram_tensor("out_oT", [OQ, L], FP, kind="ExternalOutput").ap()
    out_k = nc.dram_tensor("out_k", [OKV, L], FP, kind="ExternalOutput").ap()
    out_v = nc.dram_tensor("out_v", [L, OKV], FP, kind="ExternalOutput").ap()

    ExpF = mybir.ActivationFunctionType.Exp
    rg = [list(range(NCORES))]

    with tile.TileContext(nc) as tc:
        from contextlib import ExitStack
        pstack = ExitStack()
        with tc.tile_pool(name="cpool", bufs=1) as cpool, \
             tc.tile_pool(name="bdram", bufs=1, space="DRAM") as bdram, \
             tc.tile_pool(name="work_sb", bufs=1) as work_sb, \
             tc.tile_pool(name="at_sb", bufs=10) as at_sb, \
             tc.tile_pool(name="nm_sb", bufs=2) as nm_sb:

            acc_ps = pstack.enter_context(tc.tile_pool(name="acc_ps", bufs=2, space="PSUM"))
            mix_ps = pstack.enter_context(tc.tile_pool(name="mix_ps", bufs=3, space="PSUM"))
            ao_ps = pstack.enter_context(tc.tile_pool(name="ao_ps", bufs=1, space="PSUM"))
            vt_ps = pstack.enter_context(tc.tile_pool(name="vt_ps", bufs=1, space="PSUM"))
            # ---- resident inputs (small weights first so PE starts early) ------
            wq_sb = cpool.tile([P, NB, OQ], BF, name="wq_sb")
            nc.sync.dma_start(wq_sb, wqT.rearrange("(nb p) o -> p nb o", p=P))
            wkv_sb = cpool.tile([P, NB, 2 * OKV], BF, name="wkv_sb")
            nc.sync.dma_start(
                wkv_sb[:, :, 0:OKV], wkT.rearrange("(nb p) o -> p nb o", p=P)
            )
            nc.sync.dma_start(
                wkv_sb[:, :, OKV : 2 * OKV],
                wvT.rearrange("(nb p) o -> p nb o", p=P),
            )
            psw_sb = cpool.tile([P, P], FP, name="psw_sb")
            nc.sync.dma_start(psw_sb, pswap)
            xstack = ExitStack()
            xpool = xstack.enter_context(tc.tile_pool(name="xpool", bufs=1))
            xT_sb = xpool.tile([P, NB, L], BF, name="xT_sb")
            xTr = xT.rearrange("(nb p) l -> p nb l", p=P)
            for d in range(NB):
                nc.sync.dma_start(xT_sb[:, d, :], xTr[:, d, :])
            ctab_sb = cpool.tile([P, L], FP, name="ctab_sb")
            nc.sync.dma_start(ctab_sb, ctab)
            stab_sb = cpool.tile([P, L], FP, name="stab_sb")
            nc.sync.dma_start(stab_sb, stab)
            wo_sb = cpool.tile([P, NB, OQ], BF, name="wo_sb")
            nc.sync.dma_start(wo_sb, woT.rearrange("(nb p) o -> p nb o", p=P))
            # shifted identity: ident2[64+i, i] = 1 (transpose from base 64)
            ident2 = cpool.tile([P, HD], FP, name="ident2")
            nc.gpsimd.memset(ident2, 0.0)
            nc.gpsimd.affine_select(
                out=ident2,
                in_=ident2,
                compare_op=mybir.AluOpType.not_equal,
                fill=1.0,
                base=-HD,
                pattern=[[-1, HD]],
                channel_multiplier=1,
            )
            ones_sb = cpool.tile([1, HD], FP, name="ones_sb")
            nc.vector.memset(ones_sb, 1.0)
            # diagonal 128x128 causal mask: trimask[p, f] = 1 if f >= p else 0
            trimask = cpool.tile([P, P], BF, name="trimask")
            nc.gpsimd.memset(trimask, 1.0)
            nc.gpsimd.affine_select(
                out=trimask,
                in_=trimask,
                compare_op=mybir.AluOpType.is_ge,
                fill=0.0,
                base=0,
                pattern=[[1, P]],
                channel_multiplier=-1,
            )

            # ---- persistent activations ----------------------------------------
            qd = cpool.tile([P, HQ, L], BF, name="qd")   # rotated q, dup halves
            k_bf = cpool.tile([P, L], BF, name="k_bf")   # rotated k, dup halves
            vaug_sb = cpool.tile([P, NB, HD + 1], BF, name="vaug_sb")
            nc.vector.memset(vaug_sb[:, :, HD : HD + 1], 1.0)
            y_bf = cpool.tile([P, 2, L], BF, name="y_bf")  # attn out rows (o, l)

            # ---- helpers -------------------------------------------------------
            def project2(wA, dstA, wB, dstB):
                # kv and q0 share each xT chunk as it lands: two accumulators
                # per l-quarter drawn from the same 2-slot psum pool
                for lq in range(NCH):
                    psA = acc_ps.tile([P, CH], FP, tag="pracc", name="psA")
                    psB = acc_ps.tile([P, CH], FP, tag="pracc", name="psB")
                    for d in range(NB):
                        rhs = xT_sb[:, d, lq * CH : (lq + 1) * CH]
                        nc.tensor.matmul(
                            psA, lhsT=wA[:, d, :], rhs=rhs,
                            start=(d == 0), stop=(d == NB - 1),
                        )
                        nc.tensor.matmul(
                            psB, lhsT=wB[:, d, :], rhs=rhs,
                            start=(d == 0), stop=(d == NB - 1),
                        )
                    nc.vector.tensor_copy(
                        dstA[:, lq * CH : (lq + 1) * CH], psA
                    )
                    nc.vector.tensor_copy(
                        dstB[:, lq * CH : (lq + 1) * CH], psB
                    )

            def project(w_ap, width, dst_sb):
                # accumulate over d in l-quarters; 2-deep psum slot pipeline
                for lq in range(NCH):
                    ps = acc_ps.tile([P, CH], FP, tag="pracc", name="ps")
                    for d in range(NB):
                        nc.tensor.matmul(
                            ps[:width],
                            lhsT=w_ap[:, d, :],
                            rhs=xT_sb[:, d, lq * CH : (lq + 1) * CH],
                            start=(d == 0),
                            stop=(d == NB - 1),
                        )
                    nc.vector.tensor_copy(
                        dst_sb[:width, lq * CH : (lq + 1) * CH], ps[:width]
                    )

            def rope(src_sb, width, pslice, out_ap):
                # out = ctab*src + stab*(pairswap @ src), fully chunked so each
                # l-quarter's rope starts as soon as that quarter is projected
                for c in range(NCH):
                    cs = slice(c * CH, (c + 1) * CH)
                    swp = mix_ps.tile([P, CH], FP, tag="mps", name="swp")
                    nc.tensor.matmul(
                        swp[:width],
                        lhsT=psw_sb[:width, :width],
                        rhs=src_sb[:width, cs],
                        start=True,
                        stop=True,
                    )
                    t1 = work_sb.tile([P, CH], FP, tag="t1", bufs=2)
                    nc.vector.tensor_mul(
                        t1[:width], src_sb[:width, cs], ctab_sb[pslice, cs]
                    )
                    t2 = work_sb.tile([P, CH], FP, tag="t2", bufs=2)
                    nc.vector.tensor_mul(t2[:width], swp[:width], stab_sb[pslice, cs])
                    nc.vector.tensor_add(out_ap[:, cs], t1[:width], t2[:width])

            def project_q(t, qf=None):
                if qf is None:
                    qf = work_sb.tile([P, L], FP, tag="qf")
                    project(wq_sb[:, :, t * P : (t + 1) * P], P, qf)
                qrot = work_sb.tile([P, L], BF, tag="qrot")
                rope(qf, P, slice(0, P), qrot[:])
                # duplicate per l-quarter so early chunks unblock scores
                qdr = bdram.tile([P, L], BF, name=f"qdr{t}")
                for c in range(NCH):
                    cs = slice(c * CH, (c + 1) * CH)
                    # in-partition halves
                    nc.vector.tensor_copy(qd[0:OKV, 2 * t, cs], qrot[0:OKV, cs])
                    nc.vector.tensor_copy(
                        qd[OKV:P, 2 * t + 1, cs], qrot[OKV:P, cs]
                    )
                    # cross-partition halves via DRAM bounce
                    nc.sync.dma_start(qdr[:, cs], qrot[:, cs])
                    nc.sync.dma_start(qd[OKV:P, 2 * t, cs], qdr[0:OKV, cs])
                    nc.sync.dma_start(qd[0:OKV, 2 * t + 1, cs], qdr[OKV:P, cs])

            def attention(h):
                # chunk-group processing: 2 live [65, CH] accumulators
                for cg in range(2):
                    cset = [c for c in (2 * cg, 2 * cg + 1)]
                    pos = {
                        c: ao_ps.tile(
                            [HD + 1, CH], FP, tag=f"pog{i}", name=f"po{c}_{h}"
                        )
                        for i, c in enumerate(cset)
                    }
                    for pb in range(NB // 2):
                        mbA, mbB = 2 * pb, 2 * pb + 1
                        cs0 = mbA // 4
                        cands = [c for c in cset if c >= cs0]
                        if not cands:
                            continue
                        exs = {}
                        # scores: mbA in PE rows 0-63, mbB in rows 64-127 —
                        # the two K=64 matmuls run concurrently in the array
                        for mb, lo in ((mbA, 0), (mbB, OKV)):
                            for c in cands:
                                r = (mb - 4 * c) * P if c == cs0 else 0
                                ps_s = mix_ps.tile([P, CH], FP, tag="mps", name="ps_s")
                                nc.tensor.matmul(
                                    ps_s[:, r:CH],
                                    lhsT=k_bf[lo : lo + HD, mb * P : (mb + 1) * P],
                                    rhs=qd[
                                        lo : lo + HD, h, c * CH + r : (c + 1) * CH
                                    ],
                                    start=True,
                                    stop=True,
                                )
                                ex = at_sb.tile([P, CH], BF, tag="ex")
                                if r > 0:
                                    nc.vector.memset(ex[:, 0:r], 0.0)
                                nc.scalar.activation(
                                    ex[:, r:CH], ps_s[:, r:CH], ExpF, scale=SCALE
                                )
                                if c == cs0:
                                    nc.vector.tensor_mul(
                                        ex[:, r : r + P], ex[:, r : r + P], trimask
                                    )
                                exs[(mb, c)] = ex
                        for mb in (mbA, mbB):
                            for c in cands:
                                nc.tensor.matmul(
                                    pos[c],
                                    lhsT=vaug_sb[:, mb, :],
                                    rhs=exs[(mb, c)],
                                    start=(mb == 0),
                                    stop=(mb == 4 * c + 3),
                                )
                    # drain accumulators to SBUF fast (frees the PSUM bank),
                    # then normalize off the critical path:
                    # y = yu[0:64] * (1 / yu[64]) broadcast over rows
                    for c in cset:
                        yu = nm_sb.tile([HD + 1, CH], FP, tag="yu", bufs=4)
                        nc.vector.tensor_copy(yu, pos[c])
                        rc = nm_sb.tile([1, CH], FP, tag="rc")
                        nc.vector.reciprocal(rc, yu[HD : HD + 1, :])
                        bc = mix_ps.tile([P, CH], FP, tag="mps", name="bc")[:HD]
                        nc.tensor.matmul(
                            bc, lhsT=ones_sb, rhs=rc, start=True, stop=True
                        )
                        bcs = nm_sb.tile([HD, CH], FP, tag="bcs")
                        nc.vector.tensor_copy(bcs, bc)
                        nc.gpsimd.tensor_mul(
                            y_bf[
                                (h % 2) * HD : (h % 2) * HD + HD,
                                h // 2,
                                c * CH : (c + 1) * CH,
                            ],
                            yu[0:HD, :],
                            bcs,
                        )

            # ---- phase sequence (dataflow-overlapped) --------------------------
            # k+v packed projection (k rows 0-63, v rows 64-127) merged with
            # q0's projection so both consume each xT chunk as it arrives
            kvf = work_sb.tile([P, L], FP, tag="kvf")
            qf0 = work_sb.tile([P, L], FP, tag="qf")
            project2(wkv_sb, kvf, wq_sb[:, :, 0:P], qf0)
            krot = work_sb.tile([OKV, L], FP, tag="krot")
            rope(kvf[0:OKV], OKV, slice(0, OKV), krot[:])
            nc.sync.dma_start(out_k, krot)
            nc.vector.tensor_copy(k_bf[0:OKV, :], krot)
            kdr = bdram.tile([OKV, L], BF, name="kdr")
            nc.sync.dma_start(kdr, k_bf[0:OKV, :])
            nc.sync.dma_start(k_bf[OKV:P, :], kdr)
            project_q(0, qf=qf0)

            # v transposes -> [l, hd] tiles + f32 out (overlaps h0 scores;
            # vaug cast reads the SBUF staging copy on Pool so the PSUM bank
            # frees after a single DVE copy)
            for mb in range(NB):
                tp = vt_ps.tile([P, HD], FP, tag="vtp", name="tp")
                nc.tensor.transpose(
                    tp, kvf[OKV:P, mb * P : (mb + 1) * P], ident2[OKV:P, :]
                )
                vstg = work_sb.tile([P, HD], FP, tag="vstg", bufs=3)
                nc.vector.tensor_copy(vstg, tp)
                nc.sync.dma_start(out_v[mb * P : (mb + 1) * P, :], vstg)
                nc.gpsimd.tensor_copy(vaug_sb[:, mb, 0:HD], vstg)

            attention(0)
            project_q(1)
            xstack.close()          # frees xT's SBUF for the gathered tiles
            agpool = xstack.enter_context(tc.tile_pool(name="agpool", bufs=1))
            attention(1)

            # AG0 (heads 0-1 of every rank) — fires as soon as y tile 0 is done,
            # overlapping attention of heads 2-3
            ag_in0 = bdram.tile([P, L], BF, name="ag_in0")
            ag_out0 = bdram.tile(
                [NCORES * P, L], BF, name="ag_out0", addr_space="Shared"
            )
            nc.sync.dma_start(ag_in0[:], y_bf[:, 0, :])
            nc.gpsimd.collective_compute(
                "AllGather", mybir.AluOpType.bypass, replica_groups=rg,
                ins=[ag_in0[:].opt()], outs=[ag_out0[:].opt()],
            )
            # prefetch gathered half 0 into SBUF (reuses xT's space after q1)
            ag_all0 = agpool.tile([P, NCORES, L], BF, name="ag_all0")
            for j in range(NCORES):
                nc.sync.dma_start(ag_all0[:, j, :], ag_out0[j * P : (j + 1) * P, :])

            attention(2)
            attention(3)

            ag_in1 = bdram.tile([P, L], BF, name="ag_in1")
            ag_out1 = bdram.tile(
                [NCORES * P, L], BF, name="ag_out1", addr_space="Shared"
            )
            nc.sync.dma_start(ag_in1[:], y_bf[:, 1, :])
            nc.gpsimd.collective_compute(
                "AllGather", mybir.AluOpType.bypass, replica_groups=rg,
                ins=[ag_in1[:].opt()], outs=[ag_out1[:].opt()],
            )
            ag_all1 = agpool.tile([P, NCORES, L], BF, name="ag_all1")
            for j in range(NCORES):
                nc.sync.dma_start(ag_all1[:, j, :], ag_out1[j * P : (j + 1) * P, :])

            # ---- wo: out_T[d, l] = sum_o wo[d, o] * y_all[o, l] ----------------
            pstack.close()          # release the flat psum banks for wo
            with tc.tile_pool(name="wo_ps", bufs=2, space="PSUM") as wo_ps:
                ags = [ag_all0, ag_all1]
                for t in range(2):
                    for lhh in range(2):
                        pso = wo_ps.tile([P, HL], FP, tag="pso", name="pso")
                        for half in range(2):
                            for j in range(NCORES):
                                ob = 2 * j + half
                                for c in range(2):
                                    cs = slice(
                                        lhh * HL + c * CH, lhh * HL + (c + 1) * CH
                                    )
                                    nc.tensor.matmul(
                                        pso[:, c * CH : (c + 1) * CH],
                                        lhsT=wo_sb[:, ob, t * P : (t + 1) * P],
                                        rhs=ags[half][:, j, cs],
                                        start=(half == 0 and j == 0),
                                        stop=(half == 1 and j == NCORES - 1),
                                    )
                        of = work_sb.tile([P, HL], FP, tag="of", bufs=2)
                        nc.vector.tensor_copy(of, pso)
                        nc.sync.dma_start(
                            out_oT[t * P : (t + 1) * P, lhh * HL : (lhh + 1) * HL],
                            of,
                        )
            xstack.close()

    nc.compile()
    return nc


def _host_inputs(x, wq_w, wk_w, wv_w, wo_w):
    i = np.arange(HD // 2)
    theta = (10000.0 ** (-2.0 * i / HD)).astype(np.float64)
    pos = np.arange(1, L + 1, dtype=np.float64)
    m = theta[:, None] * pos[None, :]          # (32, L)
    ct = np.repeat(np.cos(m), 2, axis=0)       # (64, L)
    st = np.repeat(np.sin(m), 2, axis=0)
    st[0::2] *= -1.0
    ctab = np.tile(ct, (2, 1)).astype(np.float32)
    stab = np.tile(st, (2, 1)).astype(np.float32)
    pswap = np.zeros((P, P), np.float32)
    ii = np.arange(0, P, 2)
    pswap[ii, ii + 1] = 1.0
    pswap[ii + 1, ii] = 1.0

    xT = np.ascontiguousarray(x[0].T).astype(BF_NP)
    shared = {"xT": xT, "ctab": ctab, "stab": stab, "pswap": pswap}
    in_maps = []
    for c in range(NCORES):
        m_ = dict(shared)
        m_["wqT"] = np.ascontiguousarray(
            wq_w[c * OQ : (c + 1) * OQ, :].T
        ).astype(BF_NP)
        m_["wkT"] = np.ascontiguousarray(
            wk_w[c * OKV : (c + 1) * OKV, :].T
        ).astype(BF_NP)
        m_["wvT"] = np.ascontiguousarray(
            wv_w[c * OKV : (c + 1) * OKV, :].T
        ).astype(BF_NP)
        m_["woT"] = np.ascontiguousarray(
            wo_w[c * OQ : (c + 1) * OQ, :].T
        ).astype(BF_NP)
        in_maps.append(m_)
    return in_maps


def _check_causal(mask):
    mask = np.asarray(mask)
    idx = np.arange(L)
    expect = (idx[:, None] < idx[None, :]).astype(np.float32) * -1e9
    if not np.array_equal(mask.astype(np.float32), expect):
        raise ValueError(
            "kernel() hardcodes the causal mask; got a different mask tensor"
        )


def kernel(x, mask, wq_w, wk_w, wv_w, wo_w, trace=False):
    _check_causal(mask)
    x = np.asarray(x, np.float32)
    wq_w = np.asarray(wq_w, np.float32)
    wk_w = np.asarray(wk_w, np.float32)
    wv_w = np.asarray(wv_w, np.float32)
    wo_w = np.asarray(wo_w, np.float32)

    if "nc" not in _CACHE:
        _CACHE["nc"] = _build()
    nc = _CACHE["nc"]

    in_maps = _host_inputs(x, wq_w, wk_w, wv_w, wo_w)
    res = run_bass_kernel_spmd(
        nc, in_maps, core_ids=list(range(NCORES)), trace=trace
    )
    _CACHE["exec_time_ns"] = res.exec_time_ns
    _CACHE["last_results"] = res
    outs = res.results

    outT = np.concatenate([outs[c]["out_oT"] for c in range(NCORES)], axis=0)
    out = np.ascontiguousarray(outT.T)[None, :, :]                 # (1, L, D)
    k8 = np.stack(
        [np.ascontiguousarray(outs[c]["out_k"].T) for c in range(NCORES)]
    )                                                              # (8, L, 64)
    v8 = np.stack([outs[c]["out_v"] for c in range(NCORES)])       # (8, L, 64)
    k = np.repeat(k8, HQ, axis=0)[None]                            # (1, 32, L, 64)
    v = np.repeat(v8, HQ, axis=0)[None]
    return out, k, v
